# revision 4
# baseline (speedup 1.0000x reference)
"""AdaptiveCurvatureLoss on 8 TRN2 NeuronCores (Bass/Tile).

Math (verified vs the jax reference to ~5e-8 rel):
  predictions_i = sum_h w2_h * tanh(x_i*w1_h + b1_h) + b2
  mse           = mean((pred - targets)^2)
  d2y/dx2_i     = -2 * sum_h w1_h^2 w2_h * t(1-t^2),  t = tanh(x_i w1_h + b1_h)
  density_i     = 1 / (mean(3 smallest |x_i - x_j| + eps) + eps)   (self included)
  penalty       = 0.01 * (1 + 0.1*mean(density/max(density+eps))) * mean(d2^2)
  out           = (mse + penalty, mse, penalty)

Sharding: N=16384 samples row-sharded across 8 cores (2048 rows each); every
core holds a replicated copy of x for the pairwise column axis.  Per core the
kNN part processes 16 row-blocks of 128 partitions x 16384 columns:
  ACT:  |x_cols - x_row|  (f32 in, bf16 out, one 16K-wide op per block)
  DVE:  pairwise tt-min fold 16384 -> 1024 slots (bf16 2x mode), negate,
        max8 -> 8 smallest distances, top-3 -> density.
Slot-fold can only lose the 2nd/3rd-smallest on a mod-1024 column collision
(~1/1024 of rows, and density perturbs the loss at the 1e-3 level, so this is
far inside tolerance).  Per-core partial outputs (2048 densities + per-partition
sse/d2^2 sums) are combined on host - scalar epilogue only.
"""

import sys

sys.path.insert(0, "/opt/trn_rl_repo")

import numpy as np

import concourse.bass as bass
import concourse.mybir as mybir
from concourse import bacc
from concourse.bass_utils import run_bass_kernel_spmd
from concourse.tile import TileContext

N = 16384
NCORES = 8
SHARD = N // NCORES          # 2048
P = 128
NCH = SHARD // P             # 16 row-blocks / chunks per core
H = 64
EPS = 1e-8
FOLD_TO = 1024               # slots after the pairwise-min fold
F32 = mybir.dt.float32
BF16 = mybir.dt.bfloat16
ALU = mybir.AluOpType
ACTF = mybir.ActivationFunctionType


def _build():
    nc = bacc.Bacc()
    x_full = nc.declare_dram_parameter("x_full", [N], F32, isOutput=False)
    x_sh = nc.declare_dram_parameter("x_shard", [P, NCH], F32, isOutput=False)
    t_sh = nc.declare_dram_parameter("t_shard", [P, NCH], F32, isOutput=False)
    w1 = nc.declare_dram_parameter("w1", [H], F32, isOutput=False)
    b1 = nc.declare_dram_parameter("b1", [H], F32, isOutput=False)
    w2 = nc.declare_dram_parameter("w2", [H], F32, isOutput=False)
    b2 = nc.declare_dram_parameter("b2", [1], F32, isOutput=False)
    c2n = nc.declare_dram_parameter("c2n", [H], F32, isOutput=False)  # 2*w1^2*w2
    out = nc.declare_dram_parameter("out", [P, NCH + 2], F32, isOutput=True)

    with TileContext(nc) as tc:
        with (
            tc.tile_pool(name="cols", bufs=1) as colpool,
            tc.tile_pool(name="gen", bufs=2) as genpool,
            tc.tile_pool(name="fold", bufs=1) as foldpool,
            tc.tile_pool(name="small", bufs=1) as sp,
        ):
            # ---- one-time loads ----
            xcols = colpool.tile([P, N], F32)
            nc.sync.dma_start(xcols[:, :], x_full.ap().partition_broadcast(P))
            xsh = sp.tile([P, NCH], F32)
            nc.sync.dma_start(xsh[:, :], x_sh[:, :])
            tsh = sp.tile([P, NCH], F32)
            nc.sync.dma_start(tsh[:, :], t_sh[:, :])
            w1r = sp.tile([P, H], F32)
            nc.sync.dma_start(w1r[:, :], w1.ap().partition_broadcast(P))
            b1r = sp.tile([P, H], F32)
            nc.sync.dma_start(b1r[:, :], b1.ap().partition_broadcast(P))
            w2r = sp.tile([P, H], F32)
            nc.sync.dma_start(w2r[:, :], w2.ap().partition_broadcast(P))
            c2r = sp.tile([P, H], F32)
            nc.sync.dma_start(c2r[:, :], c2n.ap().partition_broadcast(P))
            b2s = sp.tile([P, 1], F32)
            nc.sync.dma_start(b2s[:, :], b2.ap().partition_broadcast(P))

            out_sb = sp.tile([P, NCH + 2], F32)
            neg_xsh = sp.tile([P, NCH], F32)
            nc.vector.tensor_scalar_mul(neg_xsh[:, :], xsh[:, :], -1.0)

            def bc_x(ap2d):  # [P, NCH] -> [P, NCH, H] (bcast along H)
                return ap2d.rearrange("p (c o) -> p c o", o=1).to_broadcast((P, NCH, H))

            def bc_h(ap2d):  # [P, H] -> [P, NCH, H] (bcast along NCH)
                return ap2d.rearrange("p (o h) -> p o h", o=1).to_broadcast((P, NCH, H))

            # ---- MLP / derivative / mse part (sample c*128+p = x_shard[p, c]) ----
            u = sp.tile([P, NCH, H], F32)
            th = sp.tile([P, NCH, H], F32)
            g = sp.tile([P, NCH, H], F32)
            nc.vector.tensor_tensor(u[:, :, :], bc_x(xsh[:, :]), bc_h(w1r[:, :]), op=ALU.mult)
            nc.vector.tensor_tensor(u[:, :, :], u[:, :, :], bc_h(b1r[:, :]), op=ALU.add)
            nc.scalar.activation(th[:, :, :], u[:, :, :], ACTF.Tanh)
            # u <- th^2 (u is dead), g <- (th^2 - 1)*th = -t(1-t^2)
            nc.scalar.activation(u[:, :, :], th[:, :, :], ACTF.Square)
            nc.vector.scalar_tensor_tensor(
                g[:, :, :], u[:, :, :], 1.0, th[:, :, :], op0=ALU.subtract, op1=ALU.mult
            )
            # pred (without b2): th * w2 summed over h  (reuse u as scratch)
            pred = sp.tile([P, NCH], F32)
            d2t = sp.tile([P, NCH], F32)
            nc.vector.tensor_tensor(u[:, :, :], th[:, :, :], bc_h(w2r[:, :]), op=ALU.mult)
            nc.vector.tensor_reduce(pred[:, :], u[:, :, :], axis=mybir.AxisListType.X, op=ALU.add)
            # d2 = sum_h (th^2-1)*th * (2 w1^2 w2)  (reuse u)
            nc.vector.tensor_tensor(u[:, :, :], g[:, :, :], bc_h(c2r[:, :]), op=ALU.mult)
            nc.vector.tensor_reduce(d2t[:, :], u[:, :, :], axis=mybir.AxisListType.X, op=ALU.add)
            # e = (pred + b2) - targets ; sse = sum(e^2) ; d2sq = sum(d2^2)
            e = sp.tile([P, NCH], F32)
            esq = sp.tile([P, NCH], F32)
            nc.vector.scalar_tensor_tensor(
                e[:, :], pred[:, :], b2s[:, 0:1], tsh[:, :], op0=ALU.add, op1=ALU.subtract
            )
            nc.scalar.activation(
                esq[:, :], e[:, :], ACTF.Square, accum_out=out_sb[:, NCH : NCH + 1]
            )
            nc.scalar.activation(
                esq[:, :], d2t[:, :], ACTF.Square, accum_out=out_sb[:, NCH + 1 : NCH + 2]
            )

            # ---- kNN density part ----
            for rb in range(NCH):
                absd = genpool.tile([P, N], BF16, tag="absd")
                nc.scalar.activation(
                    absd[:, :], xcols[:, :], ACTF.Abs,
                    bias=neg_xsh[:, rb : rb + 1], scale=1.0,
                )
                src = absd
                w = N
                while w > FOLD_TO:
                    hw = w // 2
                    dst = foldpool.tile([P, hw], BF16, tag=f"fold{hw}")
                    nc.vector.tensor_tensor(
                        dst[:, :], src[:, :hw], src[:, hw:w], op=ALU.min
                    )
                    src = dst
                    w = hw
                negf = foldpool.tile([P, FOLD_TO], BF16, tag="negf")
                nc.vector.tensor_scalar_mul(negf[:, :], src[:, :], -1.0)
                top8 = foldpool.tile([P, 8], BF16, tag="top8")
                nc.vector.max(top8[:, :], negf[:, :])
                s3 = foldpool.tile([P, 1], F32, tag="s3")
                nc.vector.tensor_reduce(
                    s3[:, :], top8[:, 0:3], axis=mybir.AxisListType.X, op=ALU.add
                )
                a3 = foldpool.tile([P, 1], F32, tag="a3")
                nc.vector.tensor_scalar(
                    a3[:, :], s3[:, :], -1.0 / 3.0, 2.0 * EPS, op0=ALU.mult, op1=ALU.add
                )
                nc.vector.reciprocal(out_sb[:, rb : rb + 1], a3[:, :])

            nc.sync.dma_start(out[:, :], out_sb[:, :])
    nc.finalize()
    return nc


_NC_CACHE = None


def _get_nc():
    global _NC_CACHE
    if _NC_CACHE is None:
        _NC_CACHE = _build()
    return _NC_CACHE


def kernel(x_input, targets, w1, b1, w2, b2, **_ignored):
    x_input = np.ascontiguousarray(x_input, dtype=np.float32)
    targets = np.ascontiguousarray(targets, dtype=np.float32)
    w1 = np.ascontiguousarray(w1, dtype=np.float32)
    b1 = np.ascontiguousarray(b1, dtype=np.float32)
    w2 = np.ascontiguousarray(w2, dtype=np.float32)
    b2 = np.ascontiguousarray(b2, dtype=np.float32)
    c2n = (2.0 * w1.astype(np.float64) ** 2 * w2.astype(np.float64)).astype(np.float32)

    in_maps = []
    for c in range(NCORES):
        xs = x_input[c * SHARD : (c + 1) * SHARD].reshape(NCH, P).T
        ts = targets[c * SHARD : (c + 1) * SHARD].reshape(NCH, P).T
        in_maps.append(
            {
                "x_full": x_input,
                "x_shard": np.ascontiguousarray(xs),
                "t_shard": np.ascontiguousarray(ts),
                "w1": w1,
                "b1": b1,
                "w2": w2,
                "b2": b2,
                "c2n": c2n,
            }
        )

    nc = _get_nc()
    res = run_bass_kernel_spmd(nc, in_maps, core_ids=list(range(NCORES)))
    outs = [r["out"] for r in res.results]

    dens = np.concatenate([o[:, :NCH].astype(np.float64).ravel() for o in outs])
    sse = sum(o[:, NCH].astype(np.float64).sum() for o in outs)
    d2sq = sum(o[:, NCH + 1].astype(np.float64).sum() for o in outs)

    mse = sse / N
    mean_densn = (dens.sum() / N) / (dens.max() + EPS)
    penalty = 0.01 * (1.0 + 0.1 * mean_densn) * (d2sq / N)
    total = mse + penalty
    return np.array([total, mse, penalty], dtype=np.float32)


if __name__ == "__main__":
    import json

    rng = np.random.default_rng(0)
    ins = {
        "x_input": rng.standard_normal(N, dtype=np.float32),
        "targets": rng.standard_normal(N, dtype=np.float32),
        "w1": (rng.standard_normal(H) * 0.5).astype(np.float32),
        "b1": np.zeros(H, np.float32),
        "w2": (rng.standard_normal(H) * 0.5).astype(np.float32),
        "b2": np.zeros(1, np.float32),
    }
    print(json.dumps([float(v) for v in kernel(**ins)]))


# revision 8
# speedup vs baseline: 1.2895x; 1.2895x over previous
"""AdaptiveCurvatureLoss on 8 TRN2 NeuronCores (Bass/Tile).

Math (verified vs the jax reference to ~5e-8 rel):
  predictions_i = sum_h w2_h * tanh(x_i*w1_h + b1_h) + b2
  mse           = mean((pred - targets)^2)
  d2y/dx2_i     = -2 * sum_h w1_h^2 w2_h * t(1-t^2),  t = tanh(x_i w1_h + b1_h)
  density_i     = 1 / (mean(3 smallest |x_i - x_j| + eps) + eps)   (self included)
  penalty       = 0.01 * (1 + 0.1*mean(density/max(density+eps))) * mean(d2^2)
  out           = (mse + penalty, mse, penalty)

Sharding: N=16384 samples row-sharded across 8 cores (2048 rows each); every
core holds a replicated copy of x for the pairwise column axis.  Per core the
kNN part processes 16 row-blocks of 128 partitions x 16384 columns:
  ACT:  |x_cols - x_row|  (f32 in, bf16 out, one 16K-wide op per block)
  DVE:  pairwise tt-min fold 16384 -> 1024 slots (bf16 2x mode), negate,
        max8 -> 8 smallest distances, top-3 -> density.
Slot-fold can only lose the 2nd/3rd-smallest on a mod-1024 column collision
(~1/1024 of rows, and density perturbs the loss at the 1e-3 level, so this is
far inside tolerance).  Per-core partial outputs (2048 densities + per-partition
sse/d2^2 sums) are combined on host - scalar epilogue only.
"""

import sys

sys.path.insert(0, "/opt/trn_rl_repo")

import numpy as np

import concourse.bass as bass
import concourse.mybir as mybir
from concourse import bacc
from concourse.bass_utils import run_bass_kernel_spmd
from concourse.tile import TileContext

N = 16384
NCORES = 8
SHARD = N // NCORES          # 2048
P = 128
NCH = SHARD // P             # 16 row-blocks / chunks per core
H = 64
EPS = 1e-8
FOLD_TO = 512                # slots after the pairwise-min fold
ACT_COLS = 13312             # columns generated by ScalarE; rest by VectorE
F32 = mybir.dt.float32
BF16 = mybir.dt.bfloat16
ALU = mybir.AluOpType
ACTF = mybir.ActivationFunctionType


def _build():
    nc = bacc.Bacc()
    x_full = nc.declare_dram_parameter("x_full", [N], F32, isOutput=False)
    x_sh = nc.declare_dram_parameter("x_shard", [P, NCH], F32, isOutput=False)
    t_sh = nc.declare_dram_parameter("t_shard", [P, NCH], F32, isOutput=False)
    w1 = nc.declare_dram_parameter("w1", [H], F32, isOutput=False)
    b1 = nc.declare_dram_parameter("b1", [H], F32, isOutput=False)
    w2 = nc.declare_dram_parameter("w2", [H], F32, isOutput=False)
    b2 = nc.declare_dram_parameter("b2", [1], F32, isOutput=False)
    c2n = nc.declare_dram_parameter("c2n", [H], F32, isOutput=False)  # 2*w1^2*w2
    out = nc.declare_dram_parameter("out", [P, NCH + 2], F32, isOutput=True)

    with TileContext(nc) as tc:
        with (
            tc.tile_pool(name="cols", bufs=1) as colpool,
            tc.tile_pool(name="gen", bufs=2) as genpool,
            tc.tile_pool(name="fold", bufs=1) as foldpool,
            tc.tile_pool(name="small", bufs=1) as sp,
        ):
            # ---- one-time loads ----
            xcols = colpool.tile([P, N], F32)
            nc.sync.dma_start(xcols[:, :], x_full.ap().partition_broadcast(P))
            xsh = sp.tile([P, NCH], F32)
            nc.sync.dma_start(xsh[:, :], x_sh[:, :])
            tsh = sp.tile([P, NCH], F32)
            nc.sync.dma_start(tsh[:, :], t_sh[:, :])
            w1r = sp.tile([P, H], F32)
            nc.sync.dma_start(w1r[:, :], w1.ap().partition_broadcast(P))
            b1r = sp.tile([P, H], F32)
            nc.sync.dma_start(b1r[:, :], b1.ap().partition_broadcast(P))
            w2r = sp.tile([P, H], F32)
            nc.sync.dma_start(w2r[:, :], w2.ap().partition_broadcast(P))
            c2r = sp.tile([P, H], F32)
            nc.sync.dma_start(c2r[:, :], c2n.ap().partition_broadcast(P))
            b2s = sp.tile([P, 1], F32)
            nc.sync.dma_start(b2s[:, :], b2.ap().partition_broadcast(P))

            out_sb = sp.tile([P, NCH + 2], F32)
            neg_xsh = sp.tile([P, NCH], F32)
            nc.vector.tensor_scalar_mul(neg_xsh[:, :], xsh[:, :], -1.0)

            def bc_x(ap2d):  # [P, NCH] -> [P, NCH, H] (bcast along H)
                return ap2d.rearrange("p (c o) -> p c o", o=1).to_broadcast((P, NCH, H))

            def bc_h(ap2d):  # [P, H] -> [P, NCH, H] (bcast along NCH)
                return ap2d.rearrange("p (o h) -> p o h", o=1).to_broadcast((P, NCH, H))

            # ---- MLP / derivative / mse part (sample c*128+p = x_shard[p, c]) ----
            u = sp.tile([P, NCH, H], F32)
            th = sp.tile([P, NCH, H], F32)
            g = sp.tile([P, NCH, H], F32)
            nc.vector.tensor_tensor(u[:, :, :], bc_x(xsh[:, :]), bc_h(w1r[:, :]), op=ALU.mult)
            nc.vector.tensor_tensor(u[:, :, :], u[:, :, :], bc_h(b1r[:, :]), op=ALU.add)
            nc.scalar.activation(th[:, :, :], u[:, :, :], ACTF.Tanh)
            # u <- th^2 (u is dead), g <- (th^2 - 1)*th = -t(1-t^2)
            nc.scalar.activation(u[:, :, :], th[:, :, :], ACTF.Square)
            nc.vector.scalar_tensor_tensor(
                g[:, :, :], u[:, :, :], 1.0, th[:, :, :], op0=ALU.subtract, op1=ALU.mult
            )
            # pred (without b2): th * w2 summed over h  (reuse u as scratch)
            pred = sp.tile([P, NCH], F32)
            d2t = sp.tile([P, NCH], F32)
            nc.vector.tensor_tensor(u[:, :, :], th[:, :, :], bc_h(w2r[:, :]), op=ALU.mult)
            nc.vector.tensor_reduce(pred[:, :], u[:, :, :], axis=mybir.AxisListType.X, op=ALU.add)
            # d2 = sum_h (th^2-1)*th * (2 w1^2 w2)  (reuse u)
            nc.vector.tensor_tensor(u[:, :, :], g[:, :, :], bc_h(c2r[:, :]), op=ALU.mult)
            nc.vector.tensor_reduce(d2t[:, :], u[:, :, :], axis=mybir.AxisListType.X, op=ALU.add)
            # e = (pred + b2) - targets ; sse = sum(e^2) ; d2sq = sum(d2^2)
            e = sp.tile([P, NCH], F32)
            esq = sp.tile([P, NCH], F32)
            nc.vector.scalar_tensor_tensor(
                e[:, :], pred[:, :], b2s[:, 0:1], tsh[:, :], op0=ALU.add, op1=ALU.subtract
            )
            nc.scalar.activation(
                esq[:, :], e[:, :], ACTF.Square, accum_out=out_sb[:, NCH : NCH + 1]
            )
            nc.scalar.activation(
                esq[:, :], d2t[:, :], ACTF.Square, accum_out=out_sb[:, NCH + 1 : NCH + 2]
            )

            # ---- kNN density part (squared-distance space) ----
            top_all = sp.tile([P, NCH, 8], BF16)
            for rb in range(NCH):
                absd = genpool.tile([P, N], BF16, tag="absd")
                # cols [0, ACT_COLS): (x_j - x_i)^2 on ScalarE
                nc.scalar.activation(
                    absd[:, :ACT_COLS], xcols[:, :ACT_COLS], ACTF.Square,
                    bias=neg_xsh[:, rb : rb + 1], scale=1.0,
                )
                # cols [ACT_COLS, N): subtract on VectorE, then square in place
                nc.vector.tensor_scalar(
                    absd[:, ACT_COLS:], xcols[:, ACT_COLS:],
                    xsh[:, rb : rb + 1], None, op0=ALU.subtract,
                )
                nc.vector.tensor_tensor(
                    absd[:, ACT_COLS:], absd[:, ACT_COLS:], absd[:, ACT_COLS:],
                    op=ALU.mult,
                )
                src = absd
                w = N
                while w > FOLD_TO:
                    hw = w // 2
                    dst = foldpool.tile([P, hw], BF16, tag=f"fold{hw}")
                    nc.vector.tensor_tensor(
                        dst[:, :], src[:, :hw], src[:, hw:w], op=ALU.min
                    )
                    src = dst
                    w = hw
                negf = foldpool.tile([P, FOLD_TO], BF16, tag="negf")
                nc.vector.tensor_scalar_mul(negf[:, :], src[:, :], -1.0)
                nc.vector.max(top_all[:, rb, :], negf[:, :])
            # batched epilogue: dens = 1/(sum(sqrt(d^2 top3))/3 + 2eps)
            nv3 = sp.tile([P, NCH, 3], F32)
            nc.vector.tensor_scalar_mul(nv3[:, :, :], top_all[:, :, 0:3], -1.0)
            dv3 = sp.tile([P, NCH, 3], F32)
            nc.scalar.activation(dv3[:, :, :], nv3[:, :, :], ACTF.Sqrt)
            s3 = sp.tile([P, NCH], F32)
            nc.vector.tensor_reduce(
                s3[:, :], dv3[:, :, :], axis=mybir.AxisListType.X, op=ALU.add
            )
            a3 = sp.tile([P, NCH], F32)
            nc.vector.tensor_scalar(
                a3[:, :], s3[:, :], 1.0 / 3.0, 2.0 * EPS, op0=ALU.mult, op1=ALU.add
            )
            nc.vector.reciprocal(out_sb[:, 0:NCH], a3[:, :])

            nc.sync.dma_start(out[:, :], out_sb[:, :])
    nc.finalize()
    return nc


_NC_CACHE = None


def _get_nc():
    global _NC_CACHE
    if _NC_CACHE is None:
        _NC_CACHE = _build()
    return _NC_CACHE


def kernel(x_input, targets, w1, b1, w2, b2, **_ignored):
    x_input = np.ascontiguousarray(x_input, dtype=np.float32)
    targets = np.ascontiguousarray(targets, dtype=np.float32)
    w1 = np.ascontiguousarray(w1, dtype=np.float32)
    b1 = np.ascontiguousarray(b1, dtype=np.float32)
    w2 = np.ascontiguousarray(w2, dtype=np.float32)
    b2 = np.ascontiguousarray(b2, dtype=np.float32)
    c2n = (2.0 * w1.astype(np.float64) ** 2 * w2.astype(np.float64)).astype(np.float32)

    in_maps = []
    for c in range(NCORES):
        xs = x_input[c * SHARD : (c + 1) * SHARD].reshape(NCH, P).T
        ts = targets[c * SHARD : (c + 1) * SHARD].reshape(NCH, P).T
        in_maps.append(
            {
                "x_full": x_input,
                "x_shard": np.ascontiguousarray(xs),
                "t_shard": np.ascontiguousarray(ts),
                "w1": w1,
                "b1": b1,
                "w2": w2,
                "b2": b2,
                "c2n": c2n,
            }
        )

    nc = _get_nc()
    res = run_bass_kernel_spmd(nc, in_maps, core_ids=list(range(NCORES)))
    outs = [r["out"] for r in res.results]

    dens = np.concatenate([o[:, :NCH].astype(np.float64).ravel() for o in outs])
    sse = sum(o[:, NCH].astype(np.float64).sum() for o in outs)
    d2sq = sum(o[:, NCH + 1].astype(np.float64).sum() for o in outs)

    mse = sse / N
    mean_densn = (dens.sum() / N) / (dens.max() + EPS)
    penalty = 0.01 * (1.0 + 0.1 * mean_densn) * (d2sq / N)
    total = mse + penalty
    return np.array([total, mse, penalty], dtype=np.float32)


if __name__ == "__main__":
    import json

    rng = np.random.default_rng(0)
    ins = {
        "x_input": rng.standard_normal(N, dtype=np.float32),
        "targets": rng.standard_normal(N, dtype=np.float32),
        "w1": (rng.standard_normal(H) * 0.5).astype(np.float32),
        "b1": np.zeros(H, np.float32),
        "w2": (rng.standard_normal(H) * 0.5).astype(np.float32),
        "b2": np.zeros(1, np.float32),
    }
    print(json.dumps([float(v) for v in kernel(**ins)]))


# revision 9
# speedup vs baseline: 2.9475x; 2.2858x over previous
"""AdaptiveCurvatureLoss on 8 TRN2 NeuronCores (Bass/Tile).

Math (verified vs the jax reference to ~5e-8 rel):
  predictions_i = sum_h w2_h * tanh(x_i*w1_h + b1_h) + b2
  mse           = mean((pred - targets)^2)
  d2y/dx2_i     = -2 * sum_h w1_h^2 w2_h * t(1-t^2),  t = tanh(x_i w1_h + b1_h)
  density_i     = 1 / (mean(3 smallest |x_i - x_j| + eps) + eps)   (self included)
  penalty       = 0.01 * (1 + 0.1*mean(density/max(density+eps))) * mean(d2^2)
  out           = (mse + penalty, mse, penalty)

Sharding: N=16384 samples row-sharded across 8 cores (2048 rows each); every
core holds a replicated copy of x for the pairwise column axis.  Per core the
kNN part processes 16 row-blocks of 128 partitions x 16384 columns:
  ACT:  |x_cols - x_row|  (f32 in, bf16 out, one 16K-wide op per block)
  DVE:  pairwise tt-min fold 16384 -> 1024 slots (bf16 2x mode), negate,
        max8 -> 8 smallest distances, top-3 -> density.
Slot-fold can only lose the 2nd/3rd-smallest on a mod-1024 column collision
(~1/1024 of rows, and density perturbs the loss at the 1e-3 level, so this is
far inside tolerance).  Per-core partial outputs (2048 densities + per-partition
sse/d2^2 sums) are combined on host - scalar epilogue only.
"""

import sys

sys.path.insert(0, "/opt/trn_rl_repo")

import numpy as np

import concourse.bass as bass
import concourse.mybir as mybir
from concourse import bacc
from concourse.bass_utils import run_bass_kernel_spmd
from concourse.tile import TileContext

N = 16384
NCORES = 8
SHARD = N // NCORES          # 2048
P = 128
NCH = SHARD // P             # 16 row-blocks / chunks per core
H = 64
EPS = 1e-8
FOLD_TO = 512                # slots after the pairwise-min fold
ACT_COLS = 14208             # columns generated by ScalarE; rest by VectorE
F32 = mybir.dt.float32
BF16 = mybir.dt.bfloat16
ALU = mybir.AluOpType
ACTF = mybir.ActivationFunctionType


def _build():
    nc = bacc.Bacc()
    x_full = nc.declare_dram_parameter("x_full", [N], F32, isOutput=False)
    x_sh = nc.declare_dram_parameter("x_shard", [P, NCH], F32, isOutput=False)
    t_sh = nc.declare_dram_parameter("t_shard", [P, NCH], F32, isOutput=False)
    w1 = nc.declare_dram_parameter("w1", [H], F32, isOutput=False)
    b1 = nc.declare_dram_parameter("b1", [H], F32, isOutput=False)
    w2 = nc.declare_dram_parameter("w2", [H], F32, isOutput=False)
    b2 = nc.declare_dram_parameter("b2", [1], F32, isOutput=False)
    c2n = nc.declare_dram_parameter("c2n", [H], F32, isOutput=False)  # 2*w1^2*w2
    out = nc.declare_dram_parameter("out", [P, NCH + 2], F32, isOutput=True)

    with TileContext(nc) as tc:
        with (
            tc.tile_pool(name="cols", bufs=1) as colpool,
            tc.tile_pool(name="gen", bufs=2) as genpool,
            tc.tile_pool(name="fold", bufs=1) as foldpool,
            tc.tile_pool(name="small", bufs=1) as sp,
        ):
            # ---- one-time loads ----
            xcols = colpool.tile([P, N], F32)
            nc.sync.dma_start(xcols[:, :], x_full.ap().partition_broadcast(P))
            xsh = sp.tile([P, NCH], F32)
            nc.sync.dma_start(xsh[:, :], x_sh[:, :])
            tsh = sp.tile([P, NCH], F32)
            nc.sync.dma_start(tsh[:, :], t_sh[:, :])
            w1r = sp.tile([P, H], F32)
            nc.sync.dma_start(w1r[:, :], w1.ap().partition_broadcast(P))
            b1r = sp.tile([P, H], F32)
            nc.sync.dma_start(b1r[:, :], b1.ap().partition_broadcast(P))
            w2r = sp.tile([P, H], F32)
            nc.sync.dma_start(w2r[:, :], w2.ap().partition_broadcast(P))
            c2r = sp.tile([P, H], F32)
            nc.sync.dma_start(c2r[:, :], c2n.ap().partition_broadcast(P))
            b2s = sp.tile([P, 1], F32)
            nc.sync.dma_start(b2s[:, :], b2.ap().partition_broadcast(P))

            out_sb = sp.tile([P, NCH + 2], F32)
            neg_xsh = sp.tile([P, NCH], F32)
            nc.vector.tensor_scalar_mul(neg_xsh[:, :], xsh[:, :], -1.0)

            def bc_x(ap2d):  # [P, NCH] -> [P, NCH, H] (bcast along H)
                return ap2d.rearrange("p (c o) -> p c o", o=1).to_broadcast((P, NCH, H))

            def bc_h(ap2d):  # [P, H] -> [P, NCH, H] (bcast along NCH)
                return ap2d.rearrange("p (o h) -> p o h", o=1).to_broadcast((P, NCH, H))

            # ---- MLP / derivative / mse part (sample c*128+p = x_shard[p, c]) ----
            u = sp.tile([P, NCH, H], F32)
            th = sp.tile([P, NCH, H], F32)
            g = sp.tile([P, NCH, H], F32)
            nc.vector.tensor_tensor(u[:, :, :], bc_x(xsh[:, :]), bc_h(w1r[:, :]), op=ALU.mult)
            nc.vector.tensor_tensor(u[:, :, :], u[:, :, :], bc_h(b1r[:, :]), op=ALU.add)
            nc.scalar.activation(th[:, :, :], u[:, :, :], ACTF.Tanh)
            # u <- th^2 (u is dead), g <- (th^2 - 1)*th = -t(1-t^2)
            nc.scalar.activation(u[:, :, :], th[:, :, :], ACTF.Square)
            nc.vector.scalar_tensor_tensor(
                g[:, :, :], u[:, :, :], 1.0, th[:, :, :], op0=ALU.subtract, op1=ALU.mult
            )
            # pred (without b2): th * w2 summed over h  (reuse u as scratch)
            pred = sp.tile([P, NCH], F32)
            d2t = sp.tile([P, NCH], F32)
            nc.vector.tensor_tensor(u[:, :, :], th[:, :, :], bc_h(w2r[:, :]), op=ALU.mult)
            nc.vector.tensor_reduce(pred[:, :], u[:, :, :], axis=mybir.AxisListType.X, op=ALU.add)
            # d2 = sum_h (th^2-1)*th * (2 w1^2 w2)  (reuse u)
            nc.vector.tensor_tensor(u[:, :, :], g[:, :, :], bc_h(c2r[:, :]), op=ALU.mult)
            nc.vector.tensor_reduce(d2t[:, :], u[:, :, :], axis=mybir.AxisListType.X, op=ALU.add)
            # e = (pred + b2) - targets ; sse = sum(e^2) ; d2sq = sum(d2^2)
            e = sp.tile([P, NCH], F32)
            esq = sp.tile([P, NCH], F32)
            nc.vector.scalar_tensor_tensor(
                e[:, :], pred[:, :], b2s[:, 0:1], tsh[:, :], op0=ALU.add, op1=ALU.subtract
            )
            nc.scalar.activation(
                esq[:, :], e[:, :], ACTF.Square, accum_out=out_sb[:, NCH : NCH + 1]
            )
            nc.scalar.activation(
                esq[:, :], d2t[:, :], ACTF.Square, accum_out=out_sb[:, NCH + 1 : NCH + 2]
            )

            # ---- kNN density part (squared-distance space) ----
            top_all = sp.tile([P, NCH, 8], BF16)
            for rb in range(NCH):
                absd = genpool.tile([P, N], BF16, tag="absd")
                # cols [0, ACT_COLS): (x_j - x_i)^2 on ScalarE
                nc.scalar.activation(
                    absd[:, :ACT_COLS], xcols[:, :ACT_COLS], ACTF.Square,
                    bias=neg_xsh[:, rb : rb + 1], scale=1.0,
                )
                # cols [ACT_COLS, N): subtract on VectorE, then square in place
                nc.vector.tensor_scalar(
                    absd[:, ACT_COLS:], xcols[:, ACT_COLS:],
                    xsh[:, rb : rb + 1], None, op0=ALU.subtract,
                )
                nc.vector.tensor_tensor(
                    absd[:, ACT_COLS:], absd[:, ACT_COLS:], absd[:, ACT_COLS:],
                    op=ALU.mult,
                )
                src = absd
                w = N
                while w > FOLD_TO:
                    hw = w // 2
                    dst = foldpool.tile([P, hw], BF16, tag=f"fold{hw}")
                    nc.vector.tensor_tensor(
                        dst[:, :], src[:, :hw], src[:, hw:w], op=ALU.min
                    )
                    src = dst
                    w = hw
                negf = foldpool.tile([P, FOLD_TO], BF16, tag="negf")
                nc.vector.tensor_scalar_mul(negf[:, :], src[:, :], -1.0)
                nc.vector.max(top_all[:, rb, :], negf[:, :])
            # batched epilogue: dens = 1/(sum(sqrt(d^2 top3))/3 + 2eps)
            nv3 = sp.tile([P, NCH, 3], F32)
            nc.vector.tensor_scalar_mul(nv3[:, :, :], top_all[:, :, 0:3], -1.0)
            dv3 = sp.tile([P, NCH, 3], F32)
            nc.scalar.activation(dv3[:, :, :], nv3[:, :, :], ACTF.Sqrt)
            s3 = sp.tile([P, NCH], F32)
            nc.vector.tensor_reduce(
                s3[:, :], dv3[:, :, :], axis=mybir.AxisListType.X, op=ALU.add
            )
            a3 = sp.tile([P, NCH], F32)
            nc.vector.tensor_scalar(
                a3[:, :], s3[:, :], 1.0 / 3.0, 2.0 * EPS, op0=ALU.mult, op1=ALU.add
            )
            nc.vector.reciprocal(out_sb[:, 0:NCH], a3[:, :])

            nc.sync.dma_start(out[:, :], out_sb[:, :])
    nc.finalize()
    return nc


_NC_CACHE = None


def _get_nc():
    global _NC_CACHE
    if _NC_CACHE is None:
        _NC_CACHE = _build()
    return _NC_CACHE


def kernel(x_input, targets, w1, b1, w2, b2, **_ignored):
    x_input = np.ascontiguousarray(x_input, dtype=np.float32)
    targets = np.ascontiguousarray(targets, dtype=np.float32)
    w1 = np.ascontiguousarray(w1, dtype=np.float32)
    b1 = np.ascontiguousarray(b1, dtype=np.float32)
    w2 = np.ascontiguousarray(w2, dtype=np.float32)
    b2 = np.ascontiguousarray(b2, dtype=np.float32)
    c2n = (2.0 * w1.astype(np.float64) ** 2 * w2.astype(np.float64)).astype(np.float32)

    in_maps = []
    for c in range(NCORES):
        xs = x_input[c * SHARD : (c + 1) * SHARD].reshape(NCH, P).T
        ts = targets[c * SHARD : (c + 1) * SHARD].reshape(NCH, P).T
        in_maps.append(
            {
                "x_full": x_input,
                "x_shard": np.ascontiguousarray(xs),
                "t_shard": np.ascontiguousarray(ts),
                "w1": w1,
                "b1": b1,
                "w2": w2,
                "b2": b2,
                "c2n": c2n,
            }
        )

    nc = _get_nc()
    res = run_bass_kernel_spmd(nc, in_maps, core_ids=list(range(NCORES)))
    outs = [r["out"] for r in res.results]

    dens = np.concatenate([o[:, :NCH].astype(np.float64).ravel() for o in outs])
    sse = sum(o[:, NCH].astype(np.float64).sum() for o in outs)
    d2sq = sum(o[:, NCH + 1].astype(np.float64).sum() for o in outs)

    mse = sse / N
    mean_densn = (dens.sum() / N) / (dens.max() + EPS)
    penalty = 0.01 * (1.0 + 0.1 * mean_densn) * (d2sq / N)
    total = mse + penalty
    return np.array([total, mse, penalty], dtype=np.float32)


if __name__ == "__main__":
    import json

    rng = np.random.default_rng(0)
    ins = {
        "x_input": rng.standard_normal(N, dtype=np.float32),
        "targets": rng.standard_normal(N, dtype=np.float32),
        "w1": (rng.standard_normal(H) * 0.5).astype(np.float32),
        "b1": np.zeros(H, np.float32),
        "w2": (rng.standard_normal(H) * 0.5).astype(np.float32),
        "b2": np.zeros(1, np.float32),
    }
    print(json.dumps([float(v) for v in kernel(**ins)]))


# revision 13
# speedup vs baseline: 3.3749x; 1.1450x over previous
"""AdaptiveCurvatureLoss on 8 TRN2 NeuronCores — bitonic-sort kNN variant.

The kNN density of a 1-D point set needs only the sorted order: each point's
two nearest neighbours lie within +-2 positions in sorted order.  So instead
of the O(N^2) pairwise matrix, every core sorts the full x (16384 values as a
[128, 128] tile) with a bitonic network:
  - compare-exchange stages along the free dim (pairs at distance j) as
    tensor_tensor min/max over strided views,
  - descending blocks handled by per-partition sign flips (host-supplied
    masks) for k >= 128, and by separate asc/desc views for k <= 64,
  - pair distances >= 128 via PE transpose (work in transposed index space).
Then neighbour diffs + a 4-candidate window give exact f32 densities.
The MLP / second-derivative / MSE parts stay row-sharded across the 8 cores
as before; host combines partial sums (scalar epilogue only).
"""

import sys

sys.path.insert(0, "/opt/trn_rl_repo")

import numpy as np

import concourse.mybir as mybir
from concourse import bacc
from concourse.bass_utils import run_bass_kernel_spmd
from concourse.tile import TileContext

N = 16384
NCORES = 8
SHARD = N // NCORES          # 2048
P = 128
W = 128                      # sort grid: [128 partitions, 128 free]
NCH = SHARD // P             # 16
H = 64
EPS = 1e-8
BIG = 1e30
F32 = mybir.dt.float32
ALU = mybir.AluOpType
ACTF = mybir.ActivationFunctionType

SGN_KS = [128 << t for t in range(7)]  # 128..8192


def _build():
    nc = bacc.Bacc()
    x_full = nc.declare_dram_parameter("x_full", [N], F32, isOutput=False)
    x_sh = nc.declare_dram_parameter("x_shard", [P, NCH], F32, isOutput=False)
    t_sh = nc.declare_dram_parameter("t_shard", [P, NCH], F32, isOutput=False)
    w1 = nc.declare_dram_parameter("w1", [H], F32, isOutput=False)
    b1 = nc.declare_dram_parameter("b1", [H], F32, isOutput=False)
    w2 = nc.declare_dram_parameter("w2", [H], F32, isOutput=False)
    b2 = nc.declare_dram_parameter("b2", [1], F32, isOutput=False)
    c2n = nc.declare_dram_parameter("c2n", [H], F32, isOutput=False)  # 2*w1^2*w2
    sgn = nc.declare_dram_parameter("signs", [P, len(SGN_KS)], F32, isOutput=False)
    sdg = nc.declare_dram_parameter("sdiag", [P, len(SGN_KS) * P], F32, isOutput=False)
    idn = nc.declare_dram_parameter("ident", [P, P], F32, isOutput=False)
    shu = nc.declare_dram_parameter("shiftu", [P, P], F32, isOutput=False)
    shd = nc.declare_dram_parameter("shiftd", [P, P], F32, isOutput=False)
    out = nc.declare_dram_parameter("out", [P, W + 2], F32, isOutput=True)

    with TileContext(nc) as tc:
        with (
            tc.tile_pool(name="sp", bufs=1) as sp,
            tc.tile_pool(name="ps", bufs=2, space="PSUM") as ps,
        ):
            # ---- loads ----
            sortA = sp.tile([P, W], F32)
            nc.sync.dma_start(sortA[:, :], x_full.ap().rearrange("(p f) -> p f", p=P))
            sortB = sp.tile([P, W], F32)
            signs = sp.tile([P, len(SGN_KS)], F32)
            nc.sync.dma_start(signs[:, :], sgn[:, :])
            sdiag = sp.tile([P, len(SGN_KS) * P], F32)
            nc.sync.dma_start(sdiag[:, :], sdg[:, :])
            ident = sp.tile([P, P], F32)
            nc.sync.dma_start(ident[:, :], idn[:, :])
            shiftu = sp.tile([P, P], F32)
            nc.sync.dma_start(shiftu[:, :], shu[:, :])
            shiftd = sp.tile([P, P], F32)
            nc.sync.dma_start(shiftd[:, :], shd[:, :])
            xsh = sp.tile([P, NCH], F32)
            nc.sync.dma_start(xsh[:, :], x_sh[:, :])
            tsh = sp.tile([P, NCH], F32)
            nc.sync.dma_start(tsh[:, :], t_sh[:, :])
            w1r = sp.tile([P, H], F32)
            nc.sync.dma_start(w1r[:, :], w1.ap().partition_broadcast(P))
            b1r = sp.tile([P, H], F32)
            nc.sync.dma_start(b1r[:, :], b1.ap().partition_broadcast(P))
            w2r = sp.tile([P, H], F32)
            nc.sync.dma_start(w2r[:, :], w2.ap().partition_broadcast(P))
            c2r = sp.tile([P, H], F32)
            nc.sync.dma_start(c2r[:, :], c2n.ap().partition_broadcast(P))
            b2s = sp.tile([P, 1], F32)
            nc.sync.dma_start(b2s[:, :], b2.ap().partition_broadcast(P))
            out_sb = sp.tile([P, W + 2], F32)

            # ---- MLP / derivative / mse (sharded; overlaps the sort) ----
            def bc_x(ap2d):
                return ap2d.rearrange("p (c o) -> p c o", o=1).to_broadcast((P, NCH, H))

            def bc_h(ap2d):
                return ap2d.rearrange("p (o h) -> p o h", o=1).to_broadcast((P, NCH, H))

            u = sp.tile([P, NCH, H], F32)
            th = sp.tile([P, NCH, H], F32)
            g = sp.tile([P, NCH, H], F32)
            nc.vector.tensor_tensor(u[:, :, :], bc_x(xsh[:, :]), bc_h(w1r[:, :]), op=ALU.mult)
            nc.vector.tensor_tensor(u[:, :, :], u[:, :, :], bc_h(b1r[:, :]), op=ALU.add)
            nc.scalar.activation(th[:, :, :], u[:, :, :], ACTF.Tanh)
            nc.scalar.activation(u[:, :, :], th[:, :, :], ACTF.Square)
            nc.vector.scalar_tensor_tensor(
                g[:, :, :], u[:, :, :], 1.0, th[:, :, :], op0=ALU.subtract, op1=ALU.mult
            )
            pred = sp.tile([P, NCH], F32)
            d2t = sp.tile([P, NCH], F32)
            nc.vector.tensor_tensor(u[:, :, :], th[:, :, :], bc_h(w2r[:, :]), op=ALU.mult)
            nc.vector.tensor_reduce(pred[:, :], u[:, :, :], axis=mybir.AxisListType.X, op=ALU.add)
            nc.vector.tensor_tensor(u[:, :, :], g[:, :, :], bc_h(c2r[:, :]), op=ALU.mult)
            nc.vector.tensor_reduce(d2t[:, :], u[:, :, :], axis=mybir.AxisListType.X, op=ALU.add)
            e = sp.tile([P, NCH], F32)
            esq = sp.tile([P, NCH], F32)
            nc.vector.scalar_tensor_tensor(
                e[:, :], pred[:, :], b2s[:, 0:1], tsh[:, :], op0=ALU.add, op1=ALU.subtract
            )
            nc.scalar.activation(
                esq[:, :], e[:, :], ACTF.Square, accum_out=out_sb[:, W : W + 1]
            )
            nc.scalar.activation(
                esq[:, :], d2t[:, :], ACTF.Square, accum_out=out_sb[:, W + 1 : W + 2]
            )

            # ---- bitonic sort ----
            def lo_hi_views(t, k, j):
                """(lo, hi, is_asc) view pairs of a [P, W] tile t for one stage."""
                if k >= W:
                    v = t[:, :].rearrange("p (c s) -> p c s", s=2 * j)
                    return [(v[:, :, 0:j], v[:, :, j : 2 * j], True)]
                v = t[:, :].rearrange("p (b r) -> p b r", r=2 * k)
                asc = v[:, :, 0:k].rearrange("p b (c s) -> p b c s", s=2 * j)
                desc = v[:, :, k : 2 * k].rearrange("p b (c s) -> p b c s", s=2 * j)
                return [
                    (asc[:, :, :, 0:j], asc[:, :, :, j : 2 * j], True),
                    (desc[:, :, :, 0:j], desc[:, :, :, j : 2 * j], False),
                ]

            # A phase's opening negate rides the post-transpose PSUM->SBUF
            # copy (ACT scale); its closing negate rides the NEXT transpose
            # (diag(signs) instead of identity: (S.x).T == x.T @ diag(S)).
            cur, alt = sortA, sortB
            pending = None  # phase whose closing un-negate is still owed

            def do_stage(k, j):
                nonlocal cur, alt
                for lo, hi, is_asc in lo_hi_views(cur, k, j):
                    alo, ahi, _ = lo_hi_views(alt, k, j)[0 if is_asc else 1]
                    nc.vector.tensor_tensor(alo, lo, hi, op=ALU.min if is_asc else ALU.max)
                    nc.vector.tensor_tensor(ahi, lo, hi, op=ALU.max if is_asc else ALU.min)
                cur, alt = alt, cur

            def do_transpose(scale_col):
                nonlocal cur, alt, pending
                pt = ps.tile([P, W], F32, tag="tpsum")
                if pending is not None:
                    ti = SGN_KS.index(pending)
                    lhs = sdiag[:, ti * P : (ti + 1) * P]
                    pending = None
                else:
                    lhs = ident[:, :]
                # real matmul: out = cur.T @ lhs (identity or diag(+-1)) —
                # exactly one nonzero per output, exact f32 transpose + sign.
                nc.tensor.matmul(pt[:, :], cur[:, :], lhs)
                if scale_col is not None:
                    nc.scalar.mul(alt[:, :], pt[:, :], signs[:, scale_col : scale_col + 1])
                else:
                    nc.scalar.copy(alt[:, :], pt[:, :])
                cur, alt = alt, cur

            for t in range(1, 15):
                k = 1 << t
                js = [k >> s for s in range(1, 20) if (k >> s) >= 1]
                if k <= 64:
                    for j in js:
                        do_stage(k, j)
                    continue
                cross = [j for j in js if j >= W]
                if cross:
                    do_transpose(None)
                    for j in cross:
                        do_stage(min(k // W, W), j // W)
                    do_transpose(SGN_KS.index(k) if k < N else None)
                else:
                    # k == 128: opening negate as a standalone ACT scaled copy
                    nc.scalar.mul(alt[:, :], cur[:, :], signs[:, 0:1])
                    cur, alt = alt, cur
                for j in js:
                    if j < W:
                        do_stage(W, j)
                if 128 <= k < N:
                    pending = k

            s = cur  # sorted ascending, idx = p*W + f

            # ---- neighbour diffs + 4-candidate window ----
            # Row-boundary values via PE shift-matrices (no slow partition-
            # shift DMAs): auxU[p] = s[p+1, col], auxD[p] = s[p-1, col].
            pu = ps.tile([P, 2], F32, tag="shpsum")
            nc.tensor.matmul(pu[:, :], shiftu[:, :], s[:, 0:2])
            auxU = sp.tile([P, 2], F32)
            nc.scalar.copy(auxU[:, :], pu[:, :])
            # engines can't address a 1-partition range at p=127; DMA can
            bigc = sp.tile([P, 2], F32)
            nc.vector.memset(bigc[:, :], BIG)
            nc.sync.dma_start(auxU[P - 1 : P, 0:2], bigc[0:1, 0:2])
            pd = ps.tile([P, 2], F32, tag="shpsum")
            nc.tensor.matmul(pd[:, :], shiftd[:, :], s[:, W - 2 : W])
            auxD = sp.tile([P, 2], F32)
            nc.scalar.copy(auxD[:, :], pd[:, :])
            dR = sp.tile([P, W + 1], F32)   # col c: R1 at idx p*W + c - 1
            d2 = sp.tile([P, W + 2], F32)   # col c: R2 at idx p*W + c - 2
            nc.vector.tensor_sub(dR[:, 1:W], s[:, 1:W], s[:, 0 : W - 1])
            nc.vector.tensor_sub(dR[:, W : W + 1], auxU[:, 0:1], s[:, W - 1 : W])
            nc.vector.tensor_sub(dR[:, 0:1], s[:, 0:1], auxD[:, 1:2])
            nc.vector.memset(dR[0:1, 0:1], BIG)
            nc.vector.tensor_sub(d2[:, 2:W], s[:, 2:W], s[:, 0 : W - 2])
            nc.vector.tensor_sub(d2[:, W : W + 1], auxU[:, 0:1], s[:, W - 2 : W - 1])
            nc.vector.tensor_sub(d2[:, W + 1 : W + 2], auxU[:, 1:2], s[:, W - 1 : W])
            nc.vector.tensor_sub(d2[:, 0:2], s[:, 0:2], auxD[:, 0:2])
            nc.vector.memset(d2[0:1, 0:2], BIG)

            ca = dR[:, 1 : W + 1]   # R1
            cb = dR[:, 0:W]         # L1
            cc = d2[:, 2 : W + 2]   # R2
            cd = d2[:, 0:W]         # L2
            ab_lo = sp.tile([P, W], F32)
            ab_hi = sp.tile([P, W], F32)
            cd_lo = sp.tile([P, W], F32)
            cd_hi = sp.tile([P, W], F32)
            nc.vector.tensor_tensor(ab_lo[:, :], ca, cb, op=ALU.min)
            nc.vector.tensor_tensor(ab_hi[:, :], ca, cb, op=ALU.max)
            nc.vector.tensor_tensor(cd_lo[:, :], cc, cd, op=ALU.min)
            nc.vector.tensor_tensor(cd_hi[:, :], cc, cd, op=ALU.max)
            m1 = sp.tile([P, W], F32)
            mm = sp.tile([P, W], F32)
            nc.vector.tensor_tensor(m1[:, :], ab_lo[:, :], cd_lo[:, :], op=ALU.min)
            nc.vector.tensor_tensor(mm[:, :], ab_lo[:, :], cd_lo[:, :], op=ALU.max)
            nc.vector.tensor_tensor(ab_lo[:, :], ab_hi[:, :], cd_hi[:, :], op=ALU.min)
            nc.vector.tensor_tensor(mm[:, :], mm[:, :], ab_lo[:, :], op=ALU.min)
            nc.vector.tensor_add(m1[:, :], m1[:, :], mm[:, :])  # d1 + d2
            a3 = sp.tile([P, W], F32)
            nc.vector.tensor_scalar(
                a3[:, :], m1[:, :], 1.0 / 3.0, 2.0 * EPS, op0=ALU.mult, op1=ALU.add
            )
            nc.vector.reciprocal(out_sb[:, 0:W], a3[:, :])

            nc.sync.dma_start(out[:, :], out_sb[:, :])
    nc.finalize()
    return nc


_NC_CACHE = None


def _get_nc():
    global _NC_CACHE
    if _NC_CACHE is None:
        _NC_CACHE = _build()
    return _NC_CACHE


def make_in_maps(x_input, targets, w1, b1, w2, b2):
    x_input = np.ascontiguousarray(x_input, dtype=np.float32)
    targets = np.ascontiguousarray(targets, dtype=np.float32)
    w1 = np.ascontiguousarray(w1, dtype=np.float32)
    b1 = np.ascontiguousarray(b1, dtype=np.float32)
    w2 = np.ascontiguousarray(w2, dtype=np.float32)
    b2 = np.ascontiguousarray(b2, dtype=np.float32)
    c2n = (2.0 * w1.astype(np.float64) ** 2 * w2.astype(np.float64)).astype(np.float32)
    pidx = np.arange(P)

    def signs_col(k):
        return np.where((pidx & (k // W)) == 0, 1.0, -1.0).astype(np.float32)

    signs = np.stack([signs_col(k) for k in SGN_KS], axis=1).astype(np.float32)
    sdiag = np.concatenate(
        [np.diag(signs[:, t]) for t in range(len(SGN_KS))], axis=1
    ).astype(np.float32)
    identity = np.eye(P, dtype=np.float32)
    shiftu = np.eye(P, P, -1, dtype=np.float32)  # auxU[m] = s[m+1]
    shiftd = np.eye(P, P, 1, dtype=np.float32)   # auxD[m] = s[m-1]
    in_maps = []
    for c in range(NCORES):
        xs = x_input[c * SHARD : (c + 1) * SHARD].reshape(NCH, P).T
        ts = targets[c * SHARD : (c + 1) * SHARD].reshape(NCH, P).T
        in_maps.append(
            {
                "x_full": x_input,
                "x_shard": np.ascontiguousarray(xs),
                "t_shard": np.ascontiguousarray(ts),
                "w1": w1,
                "b1": b1,
                "w2": w2,
                "b2": b2,
                "c2n": c2n,
                "signs": signs,
                "sdiag": sdiag,
                "ident": identity,
                "shiftu": shiftu,
                "shiftd": shiftd,
            }
        )
    return in_maps


def kernel(x_input, targets, w1, b1, w2, b2, **_ignored):
    in_maps = make_in_maps(x_input, targets, w1, b1, w2, b2)
    nc = _get_nc()
    res = run_bass_kernel_spmd(nc, in_maps, core_ids=list(range(NCORES)))
    outs = [r["out"] for r in res.results]

    dens = outs[0][:, :W].astype(np.float64).ravel()
    sse = sum(o[:, W].astype(np.float64).sum() for o in outs)
    d2sq = sum(o[:, W + 1].astype(np.float64).sum() for o in outs)

    mse = sse / N
    mean_densn = (dens.sum() / N) / (dens.max() + EPS)
    penalty = 0.01 * (1.0 + 0.1 * mean_densn) * (d2sq / N)
    total = mse + penalty
    return np.array([total, mse, penalty], dtype=np.float32)


# revision 18
# speedup vs baseline: 3.5702x; 1.0579x over previous
"""AdaptiveCurvatureLoss on 8 TRN2 NeuronCores — bitonic-sort kNN variant.

The kNN density of a 1-D point set needs only the sorted order: each point's
two nearest neighbours lie within +-2 positions in sorted order.  So instead
of the O(N^2) pairwise matrix, every core sorts the full x (16384 values as a
[128, 128] tile) with a bitonic network:
  - compare-exchange stages along the free dim (pairs at distance j) as
    tensor_tensor min/max over strided views,
  - descending blocks handled by per-partition sign flips (host-supplied
    masks) for k >= 128, and by separate asc/desc views for k <= 64,
  - pair distances >= 128 via PE transpose (work in transposed index space).
Then neighbour diffs + a 4-candidate window give exact f32 densities.
The MLP / second-derivative / MSE parts stay row-sharded across the 8 cores
as before; host combines partial sums (scalar epilogue only).
"""

import sys

sys.path.insert(0, "/opt/trn_rl_repo")

import numpy as np

import concourse.mybir as mybir
from concourse import bacc
from concourse.bass_utils import run_bass_kernel_spmd
from concourse.tile import TileContext

N = 16384
NCORES = 8
SHARD = N // NCORES          # 2048
P = 128
W = 128                      # sort grid: [128 partitions, 128 free]
NCH = SHARD // P             # 16
H = 64
EPS = 1e-8
BIG = 1e30
F32 = mybir.dt.float32
ALU = mybir.AluOpType
ACTF = mybir.ActivationFunctionType

SGN_KS = [128 << t for t in range(7)]  # 128..8192


def _build():
    nc = bacc.Bacc()
    x_full = nc.declare_dram_parameter("x_full", [N], F32, isOutput=False)
    x_sh = nc.declare_dram_parameter("x_shard", [P, NCH], F32, isOutput=False)
    t_sh = nc.declare_dram_parameter("t_shard", [P, NCH], F32, isOutput=False)
    w1 = nc.declare_dram_parameter("w1", [H], F32, isOutput=False)
    b1 = nc.declare_dram_parameter("b1", [H], F32, isOutput=False)
    w2 = nc.declare_dram_parameter("w2", [H], F32, isOutput=False)
    b2 = nc.declare_dram_parameter("b2", [1], F32, isOutput=False)
    c2n = nc.declare_dram_parameter("c2n", [H], F32, isOutput=False)  # 2*w1^2*w2
    sgn = nc.declare_dram_parameter("signs", [P, len(SGN_KS)], F32, isOutput=False)
    idn = nc.declare_dram_parameter("ident", [P, P], F32, isOutput=False)
    shu = nc.declare_dram_parameter("shiftu", [P, P], F32, isOutput=False)
    shd = nc.declare_dram_parameter("shiftd", [P, P], F32, isOutput=False)
    out = nc.declare_dram_parameter("out", [P, W + 2], F32, isOutput=True)

    with TileContext(nc) as tc:
        with (
            tc.tile_pool(name="sp", bufs=1) as sp,
            tc.tile_pool(name="ps", bufs=2, space="PSUM") as ps,
        ):
            # ---- loads ----
            sortA = sp.tile([P, W], F32)
            nc.sync.dma_start(sortA[:, :], x_full.ap().rearrange("(p f) -> p f", p=P))
            sortB = sp.tile([P, W], F32)
            signs = sp.tile([P, len(SGN_KS)], F32)
            nc.sync.dma_start(signs[:, :], sgn[:, :])
            ident = sp.tile([P, P], F32)
            nc.sync.dma_start(ident[:, :], idn[:, :])
            shiftu = sp.tile([P, P], F32)
            nc.sync.dma_start(shiftu[:, :], shu[:, :])
            shiftd = sp.tile([P, P], F32)
            nc.sync.dma_start(shiftd[:, :], shd[:, :])
            xsh = sp.tile([P, NCH], F32)
            nc.sync.dma_start(xsh[:, :], x_sh[:, :])
            tsh = sp.tile([P, NCH], F32)
            nc.sync.dma_start(tsh[:, :], t_sh[:, :])
            w1r = sp.tile([P, H], F32)
            nc.sync.dma_start(w1r[:, :], w1.ap().partition_broadcast(P))
            b1r = sp.tile([P, H], F32)
            nc.sync.dma_start(b1r[:, :], b1.ap().partition_broadcast(P))
            w2r = sp.tile([P, H], F32)
            nc.sync.dma_start(w2r[:, :], w2.ap().partition_broadcast(P))
            c2r = sp.tile([P, H], F32)
            nc.sync.dma_start(c2r[:, :], c2n.ap().partition_broadcast(P))
            b2s = sp.tile([P, 1], F32)
            nc.sync.dma_start(b2s[:, :], b2.ap().partition_broadcast(P))
            out_sb = sp.tile([P, W + 2], F32)

            # ---- MLP / derivative / mse (sharded; overlaps the sort) ----
            def bc_x(ap2d):
                return ap2d.rearrange("p (c o) -> p c o", o=1).to_broadcast((P, NCH, H))

            def bc_h(ap2d):
                return ap2d.rearrange("p (o h) -> p o h", o=1).to_broadcast((P, NCH, H))

            u = sp.tile([P, NCH, H], F32)
            th = sp.tile([P, NCH, H], F32)
            g = sp.tile([P, NCH, H], F32)
            nc.vector.tensor_tensor(u[:, :, :], bc_x(xsh[:, :]), bc_h(w1r[:, :]), op=ALU.mult)
            nc.vector.tensor_tensor(u[:, :, :], u[:, :, :], bc_h(b1r[:, :]), op=ALU.add)
            nc.scalar.activation(th[:, :, :], u[:, :, :], ACTF.Tanh)
            nc.scalar.activation(u[:, :, :], th[:, :, :], ACTF.Square)
            nc.vector.scalar_tensor_tensor(
                g[:, :, :], u[:, :, :], 1.0, th[:, :, :], op0=ALU.subtract, op1=ALU.mult
            )
            pred = sp.tile([P, NCH], F32)
            d2t = sp.tile([P, NCH], F32)
            nc.vector.tensor_tensor(u[:, :, :], th[:, :, :], bc_h(w2r[:, :]), op=ALU.mult)
            nc.vector.tensor_reduce(pred[:, :], u[:, :, :], axis=mybir.AxisListType.X, op=ALU.add)
            nc.vector.tensor_tensor(u[:, :, :], g[:, :, :], bc_h(c2r[:, :]), op=ALU.mult)
            nc.vector.tensor_reduce(d2t[:, :], u[:, :, :], axis=mybir.AxisListType.X, op=ALU.add)
            e = sp.tile([P, NCH], F32)
            esq = sp.tile([P, NCH], F32)
            nc.vector.scalar_tensor_tensor(
                e[:, :], pred[:, :], b2s[:, 0:1], tsh[:, :], op0=ALU.add, op1=ALU.subtract
            )
            nc.scalar.activation(
                esq[:, :], e[:, :], ACTF.Square, accum_out=out_sb[:, W : W + 1]
            )
            nc.scalar.activation(
                esq[:, :], d2t[:, :], ACTF.Square, accum_out=out_sb[:, W + 1 : W + 2]
            )

            # ---- bitonic sort ----
            def lo_hi_views(t, k, j):
                """(lo, hi, is_asc) view pairs of a [P, W] tile t for one stage."""
                if k >= W:
                    v = t[:, :].rearrange("p (c s) -> p c s", s=2 * j)
                    return [(v[:, :, 0:j], v[:, :, j : 2 * j], True)]
                v = t[:, :].rearrange("p (b r) -> p b r", r=2 * k)
                asc = v[:, :, 0:k].rearrange("p b (c s) -> p b c s", s=2 * j)
                desc = v[:, :, k : 2 * k].rearrange("p b (c s) -> p b c s", s=2 * j)
                return [
                    (asc[:, :, :, 0:j], asc[:, :, :, j : 2 * j], True),
                    (desc[:, :, :, 0:j], desc[:, :, :, j : 2 * j], False),
                ]

            # A phase's opening negate rides the post-transpose PSUM->SBUF
            # copy (ACT scale); its closing negate is a cheap in-place DVE
            # tensor_scalar.  Transposes use the fast PE is_transpose path.
            cur, alt = sortA, sortB

            def do_stage(k, j):
                nonlocal cur, alt
                for lo, hi, is_asc in lo_hi_views(cur, k, j):
                    alo, ahi, _ = lo_hi_views(alt, k, j)[0 if is_asc else 1]
                    nc.vector.tensor_tensor(alo, lo, hi, op=ALU.min if is_asc else ALU.max)
                    nc.vector.tensor_tensor(ahi, lo, hi, op=ALU.max if is_asc else ALU.min)
                cur, alt = alt, cur

            def do_transpose(scale_col):
                nonlocal cur, alt
                pt = ps.tile([P, W], F32, tag="tpsum")
                nc.tensor.transpose(pt[:, :], cur[:, :], ident[:, :])
                if scale_col is not None:
                    nc.scalar.mul(alt[:, :], pt[:, :], signs[:, scale_col : scale_col + 1])
                else:
                    nc.scalar.copy(alt[:, :], pt[:, :])
                cur, alt = alt, cur

            for t in range(1, 15):
                k = 1 << t
                js = [k >> s for s in range(1, 20) if (k >> s) >= 1]
                if k <= 64:
                    for j in js:
                        do_stage(k, j)
                    continue
                cross = [j for j in js if j >= W]
                if cross:
                    do_transpose(None)
                    for j in cross:
                        do_stage(min(k // W, W), j // W)
                    do_transpose(SGN_KS.index(k) if k < N else None)
                else:
                    # k == 128: opening negate as a standalone ACT scaled copy
                    nc.scalar.mul(alt[:, :], cur[:, :], signs[:, 0:1])
                    cur, alt = alt, cur
                for j in js:
                    if j < W:
                        do_stage(W, j)
                if 128 <= k < N:
                    # closing un-negate, in place on the DVE
                    col = SGN_KS.index(k)
                    nc.vector.tensor_scalar(
                        cur[:, :], cur[:, :], signs[:, col : col + 1], None, op0=ALU.mult
                    )

            s = cur  # sorted ascending, idx = p*W + f

            # ---- neighbour diffs + 4-candidate window ----
            # Row-boundary values via PE shift-matrices (no slow partition-
            # shift DMAs): auxU[p] = s[p+1, col], auxD[p] = s[p-1, col].
            pu = ps.tile([P, 2], F32, tag="shpsum")
            nc.tensor.matmul(pu[:, :], shiftu[:, :], s[:, 0:2])
            auxU = sp.tile([P, 2], F32)
            nc.scalar.copy(auxU[:, :], pu[:, :])
            # engines can't address a 1-partition range at p=127; DMA can
            bigc = sp.tile([P, 2], F32)
            nc.vector.memset(bigc[:, :], BIG)
            nc.sync.dma_start(auxU[P - 1 : P, 0:2], bigc[0:1, 0:2])
            pd = ps.tile([P, 2], F32, tag="shpsum")
            nc.tensor.matmul(pd[:, :], shiftd[:, :], s[:, W - 2 : W])
            auxD = sp.tile([P, 2], F32)
            nc.scalar.copy(auxD[:, :], pd[:, :])
            dR = sp.tile([P, W + 1], F32)   # col c: R1 at idx p*W + c - 1
            d2 = sp.tile([P, W + 2], F32)   # col c: R2 at idx p*W + c - 2
            nc.vector.tensor_sub(dR[:, 1:W], s[:, 1:W], s[:, 0 : W - 1])
            nc.vector.tensor_sub(dR[:, W : W + 1], auxU[:, 0:1], s[:, W - 1 : W])
            nc.vector.tensor_sub(dR[:, 0:1], s[:, 0:1], auxD[:, 1:2])
            nc.vector.memset(dR[0:1, 0:1], BIG)
            nc.vector.tensor_sub(d2[:, 2:W], s[:, 2:W], s[:, 0 : W - 2])
            nc.vector.tensor_sub(d2[:, W : W + 1], auxU[:, 0:1], s[:, W - 2 : W - 1])
            nc.vector.tensor_sub(d2[:, W + 1 : W + 2], auxU[:, 1:2], s[:, W - 1 : W])
            nc.vector.tensor_sub(d2[:, 0:2], s[:, 0:2], auxD[:, 0:2])
            nc.vector.memset(d2[0:1, 0:2], BIG)

            ca = dR[:, 1 : W + 1]   # R1
            cb = dR[:, 0:W]         # L1
            cc = d2[:, 2 : W + 2]   # R2
            cd = d2[:, 0:W]         # L2
            ab_lo = sp.tile([P, W], F32)
            ab_hi = sp.tile([P, W], F32)
            cd_lo = sp.tile([P, W], F32)
            cd_hi = sp.tile([P, W], F32)
            nc.vector.tensor_tensor(ab_lo[:, :], ca, cb, op=ALU.min)
            nc.vector.tensor_tensor(ab_hi[:, :], ca, cb, op=ALU.max)
            nc.vector.tensor_tensor(cd_lo[:, :], cc, cd, op=ALU.min)
            nc.vector.tensor_tensor(cd_hi[:, :], cc, cd, op=ALU.max)
            m1 = sp.tile([P, W], F32)
            mm = sp.tile([P, W], F32)
            nc.vector.tensor_tensor(m1[:, :], ab_lo[:, :], cd_lo[:, :], op=ALU.min)
            nc.vector.tensor_tensor(mm[:, :], ab_lo[:, :], cd_lo[:, :], op=ALU.max)
            nc.vector.tensor_tensor(ab_lo[:, :], ab_hi[:, :], cd_hi[:, :], op=ALU.min)
            nc.vector.tensor_tensor(mm[:, :], mm[:, :], ab_lo[:, :], op=ALU.min)
            nc.vector.tensor_add(m1[:, :], m1[:, :], mm[:, :])  # d1 + d2
            a3 = sp.tile([P, W], F32)
            nc.vector.tensor_scalar(
                a3[:, :], m1[:, :], 1.0 / 3.0, 2.0 * EPS, op0=ALU.mult, op1=ALU.add
            )
            nc.vector.reciprocal(out_sb[:, 0:W], a3[:, :])

            nc.sync.dma_start(out[:, :], out_sb[:, :])
    nc.finalize()
    return nc


_NC_CACHE = None


def _get_nc():
    global _NC_CACHE
    if _NC_CACHE is None:
        _NC_CACHE = _build()
    return _NC_CACHE


def make_in_maps(x_input, targets, w1, b1, w2, b2):
    x_input = np.ascontiguousarray(x_input, dtype=np.float32)
    targets = np.ascontiguousarray(targets, dtype=np.float32)
    w1 = np.ascontiguousarray(w1, dtype=np.float32)
    b1 = np.ascontiguousarray(b1, dtype=np.float32)
    w2 = np.ascontiguousarray(w2, dtype=np.float32)
    b2 = np.ascontiguousarray(b2, dtype=np.float32)
    c2n = (2.0 * w1.astype(np.float64) ** 2 * w2.astype(np.float64)).astype(np.float32)
    pidx = np.arange(P)

    def signs_col(k):
        return np.where((pidx & (k // W)) == 0, 1.0, -1.0).astype(np.float32)

    signs = np.stack([signs_col(k) for k in SGN_KS], axis=1).astype(np.float32)
    identity = np.eye(P, dtype=np.float32)
    shiftu = np.eye(P, P, -1, dtype=np.float32)  # auxU[m] = s[m+1]
    shiftd = np.eye(P, P, 1, dtype=np.float32)   # auxD[m] = s[m-1]
    in_maps = []
    for c in range(NCORES):
        xs = x_input[c * SHARD : (c + 1) * SHARD].reshape(NCH, P).T
        ts = targets[c * SHARD : (c + 1) * SHARD].reshape(NCH, P).T
        in_maps.append(
            {
                "x_full": x_input,
                "x_shard": np.ascontiguousarray(xs),
                "t_shard": np.ascontiguousarray(ts),
                "w1": w1,
                "b1": b1,
                "w2": w2,
                "b2": b2,
                "c2n": c2n,
                "signs": signs,
                "ident": identity,
                "shiftu": shiftu,
                "shiftd": shiftd,
            }
        )
    return in_maps


def kernel(x_input, targets, w1, b1, w2, b2, **_ignored):
    in_maps = make_in_maps(x_input, targets, w1, b1, w2, b2)
    nc = _get_nc()
    res = run_bass_kernel_spmd(nc, in_maps, core_ids=list(range(NCORES)))
    outs = [r["out"] for r in res.results]

    dens = outs[0][:, :W].astype(np.float64).ravel()
    sse = sum(o[:, W].astype(np.float64).sum() for o in outs)
    d2sq = sum(o[:, W + 1].astype(np.float64).sum() for o in outs)

    mse = sse / N
    mean_densn = (dens.sum() / N) / (dens.max() + EPS)
    penalty = 0.01 * (1.0 + 0.1 * mean_densn) * (d2sq / N)
    total = mse + penalty
    return np.array([total, mse, penalty], dtype=np.float32)


# revision 20
# speedup vs baseline: 3.6527x; 1.0231x over previous
"""AdaptiveCurvatureLoss on 8 TRN2 NeuronCores — bitonic-sort kNN variant.

The kNN density of a 1-D point set needs only the sorted order: each point's
two nearest neighbours lie within +-2 positions in sorted order.  So instead
of the O(N^2) pairwise matrix, every core sorts the full x (16384 values as a
[128, 128] tile) with a bitonic network:
  - compare-exchange stages along the free dim (pairs at distance j) as
    tensor_tensor min/max over strided views,
  - descending blocks handled by per-partition sign flips (host-supplied
    masks) for k >= 128, and by separate asc/desc views for k <= 64,
  - pair distances >= 128 via PE transpose (work in transposed index space).
Then neighbour diffs + a 4-candidate window give exact f32 densities.
The MLP / second-derivative / MSE parts stay row-sharded across the 8 cores
as before; host combines partial sums (scalar epilogue only).
"""

import sys

sys.path.insert(0, "/opt/trn_rl_repo")

import numpy as np

import concourse.mybir as mybir
from concourse import bacc
from concourse.bass_utils import run_bass_kernel_spmd
from concourse.tile import TileContext

N = 16384
NCORES = 8
SHARD = N // NCORES          # 2048
P = 128
W = 128                      # sort grid: [128 partitions, 128 free]
NCH = SHARD // P             # 16
H = 64
EPS = 1e-8
BIG = 1e30
F32 = mybir.dt.float32
ALU = mybir.AluOpType
ACTF = mybir.ActivationFunctionType

SGN_KS = [128 << t for t in range(7)]  # 128..8192


def _build():
    nc = bacc.Bacc()
    x_full = nc.declare_dram_parameter("x_full", [N], F32, isOutput=False)
    x_sh = nc.declare_dram_parameter("x_shard", [P, NCH], F32, isOutput=False)
    t_sh = nc.declare_dram_parameter("t_shard", [P, NCH], F32, isOutput=False)
    w1 = nc.declare_dram_parameter("w1", [H], F32, isOutput=False)
    b1 = nc.declare_dram_parameter("b1", [H], F32, isOutput=False)
    w2 = nc.declare_dram_parameter("w2", [H], F32, isOutput=False)
    b2 = nc.declare_dram_parameter("b2", [1], F32, isOutput=False)
    c2n = nc.declare_dram_parameter("c2n", [H], F32, isOutput=False)  # 2*w1^2*w2
    sgn = nc.declare_dram_parameter("signs", [P, len(SGN_KS)], F32, isOutput=False)
    idn = nc.declare_dram_parameter("ident", [P, P], F32, isOutput=False)
    shu = nc.declare_dram_parameter("shiftu", [P, P], F32, isOutput=False)
    shd = nc.declare_dram_parameter("shiftd", [P, P], F32, isOutput=False)
    out = nc.declare_dram_parameter("out", [P, W + 2], F32, isOutput=True)

    with TileContext(nc) as tc:
        with (
            tc.tile_pool(name="sp", bufs=1) as sp,
            tc.tile_pool(name="ps", bufs=2, space="PSUM") as ps,
        ):
            # ---- loads ----
            sortA = sp.tile([P, W], F32)
            nc.sync.dma_start(sortA[:, :], x_full.ap().rearrange("(p f) -> p f", p=P))
            sortB = sp.tile([P, W], F32)
            signs = sp.tile([P, len(SGN_KS)], F32)
            nc.sync.dma_start(signs[:, :], sgn[:, :])
            ident = sp.tile([P, P], F32)
            nc.sync.dma_start(ident[:, :], idn[:, :])
            # non-sort-critical loads go on the gpsimd DMA queue so they don't
            # serialize behind the sort's sync-queue DMAs
            shiftu = sp.tile([P, P], F32)
            nc.gpsimd.dma_start(shiftu[:, :], shu[:, :])
            shiftd = sp.tile([P, P], F32)
            nc.gpsimd.dma_start(shiftd[:, :], shd[:, :])
            xsh = sp.tile([P, NCH], F32)
            nc.gpsimd.dma_start(xsh[:, :], x_sh[:, :])
            tsh = sp.tile([P, NCH], F32)
            nc.gpsimd.dma_start(tsh[:, :], t_sh[:, :])
            w1r = sp.tile([P, H], F32)
            nc.gpsimd.dma_start(w1r[:, :], w1.ap().partition_broadcast(P))
            b1r = sp.tile([P, H], F32)
            nc.gpsimd.dma_start(b1r[:, :], b1.ap().partition_broadcast(P))
            w2r = sp.tile([P, H], F32)
            nc.gpsimd.dma_start(w2r[:, :], w2.ap().partition_broadcast(P))
            c2r = sp.tile([P, H], F32)
            nc.gpsimd.dma_start(c2r[:, :], c2n.ap().partition_broadcast(P))
            b2s = sp.tile([P, 1], F32)
            nc.gpsimd.dma_start(b2s[:, :], b2.ap().partition_broadcast(P))
            out_sb = sp.tile([P, W + 2], F32)
            # BIG row for the auxU boundary, prepared up front
            auxU = sp.tile([P, 2], F32)
            bigc = sp.tile([P, 2], F32)
            nc.vector.memset(bigc[:, :], BIG)
            nc.gpsimd.dma_start(auxU[P - 1 : P, 0:2], bigc[0:1, 0:2])

            # ---- MLP / derivative / mse (sharded; overlaps the sort) ----
            def bc_x(ap2d):
                return ap2d.rearrange("p (c o) -> p c o", o=1).to_broadcast((P, NCH, H))

            def bc_h(ap2d):
                return ap2d.rearrange("p (o h) -> p o h", o=1).to_broadcast((P, NCH, H))

            u = sp.tile([P, NCH, H], F32)
            th = sp.tile([P, NCH, H], F32)
            g = sp.tile([P, NCH, H], F32)
            nc.vector.tensor_tensor(u[:, :, :], bc_x(xsh[:, :]), bc_h(w1r[:, :]), op=ALU.mult)
            nc.vector.tensor_tensor(u[:, :, :], u[:, :, :], bc_h(b1r[:, :]), op=ALU.add)
            nc.scalar.activation(th[:, :, :], u[:, :, :], ACTF.Tanh)
            nc.scalar.activation(u[:, :, :], th[:, :, :], ACTF.Square)
            nc.vector.scalar_tensor_tensor(
                g[:, :, :], u[:, :, :], 1.0, th[:, :, :], op0=ALU.subtract, op1=ALU.mult
            )
            pred = sp.tile([P, NCH], F32)
            d2t = sp.tile([P, NCH], F32)
            nc.vector.tensor_tensor(u[:, :, :], th[:, :, :], bc_h(w2r[:, :]), op=ALU.mult)
            nc.vector.tensor_reduce(pred[:, :], u[:, :, :], axis=mybir.AxisListType.X, op=ALU.add)
            nc.vector.tensor_tensor(u[:, :, :], g[:, :, :], bc_h(c2r[:, :]), op=ALU.mult)
            nc.vector.tensor_reduce(d2t[:, :], u[:, :, :], axis=mybir.AxisListType.X, op=ALU.add)
            e = sp.tile([P, NCH], F32)
            esq = sp.tile([P, NCH], F32)
            nc.vector.scalar_tensor_tensor(
                e[:, :], pred[:, :], b2s[:, 0:1], tsh[:, :], op0=ALU.add, op1=ALU.subtract
            )
            nc.scalar.activation(
                esq[:, :], e[:, :], ACTF.Square, accum_out=out_sb[:, W : W + 1]
            )
            nc.scalar.activation(
                esq[:, :], d2t[:, :], ACTF.Square, accum_out=out_sb[:, W + 1 : W + 2]
            )

            # ---- bitonic sort ----
            def lo_hi_views(t, k, j):
                """(lo, hi, is_asc) view pairs of a [P, W] tile t for one stage."""
                if k >= W:
                    v = t[:, :].rearrange("p (c s) -> p c s", s=2 * j)
                    return [(v[:, :, 0:j], v[:, :, j : 2 * j], True)]
                v = t[:, :].rearrange("p (b r) -> p b r", r=2 * k)
                asc = v[:, :, 0:k].rearrange("p b (c s) -> p b c s", s=2 * j)
                desc = v[:, :, k : 2 * k].rearrange("p b (c s) -> p b c s", s=2 * j)
                return [
                    (asc[:, :, :, 0:j], asc[:, :, :, j : 2 * j], True),
                    (desc[:, :, :, 0:j], desc[:, :, :, j : 2 * j], False),
                ]

            # A phase's opening negate rides the post-transpose PSUM->SBUF
            # copy (ACT scale); its closing negate is a cheap in-place DVE
            # tensor_scalar.  Transposes use the fast PE is_transpose path.
            cur, alt = sortA, sortB

            def do_stage(k, j):
                nonlocal cur, alt
                for lo, hi, is_asc in lo_hi_views(cur, k, j):
                    alo, ahi, _ = lo_hi_views(alt, k, j)[0 if is_asc else 1]
                    nc.vector.tensor_tensor(alo, lo, hi, op=ALU.min if is_asc else ALU.max)
                    nc.vector.tensor_tensor(ahi, lo, hi, op=ALU.max if is_asc else ALU.min)
                cur, alt = alt, cur

            def do_transpose(scale_col):
                nonlocal cur, alt
                pt = ps.tile([P, W], F32, tag="tpsum")
                nc.tensor.transpose(pt[:, :], cur[:, :], ident[:, :])
                if scale_col is not None:
                    nc.scalar.mul(alt[:, :], pt[:, :], signs[:, scale_col : scale_col + 1])
                else:
                    nc.scalar.copy(alt[:, :], pt[:, :])
                cur, alt = alt, cur

            for t in range(1, 15):
                k = 1 << t
                js = [k >> s for s in range(1, 20) if (k >> s) >= 1]
                if k <= 64:
                    for j in js:
                        do_stage(k, j)
                    continue
                cross = [j for j in js if j >= W]
                if cross:
                    do_transpose(None)
                    for j in cross:
                        do_stage(min(k // W, W), j // W)
                    do_transpose(SGN_KS.index(k) if k < N else None)
                else:
                    # k == 128: opening negate as a standalone ACT scaled copy
                    nc.scalar.mul(alt[:, :], cur[:, :], signs[:, 0:1])
                    cur, alt = alt, cur
                for j in js:
                    if j < W:
                        do_stage(W, j)
                if 128 <= k < N:
                    # closing un-negate, in place on the DVE
                    col = SGN_KS.index(k)
                    nc.vector.tensor_scalar(
                        cur[:, :], cur[:, :], signs[:, col : col + 1], None, op0=ALU.mult
                    )

            s = cur  # sorted ascending, idx = p*W + f

            # ---- neighbour diffs + 4-candidate window ----
            # Row-boundary values via PE shift-matrices (no slow partition-
            # shift DMAs): auxU[p] = s[p+1, col], auxD[p] = s[p-1, col].
            pu = ps.tile([P, 2], F32, tag="shpsum")
            nc.tensor.matmul(pu[:, :], shiftu[:, :], s[:, 0:2])
            # partition 127 was pre-filled with BIG via DMA (engines can't
            # address a 1-partition range at p=127); copy only 0..126 here
            nc.scalar.copy(auxU[0 : P - 1, :], pu[0 : P - 1, :])
            pd = ps.tile([P, 2], F32, tag="shpsum")
            nc.tensor.matmul(pd[:, :], shiftd[:, :], s[:, W - 2 : W])
            auxD = sp.tile([P, 2], F32)
            nc.scalar.copy(auxD[:, :], pd[:, :])
            dR = sp.tile([P, W + 1], F32)   # col c: R1 at idx p*W + c - 1
            d2 = sp.tile([P, W + 2], F32)   # col c: R2 at idx p*W + c - 2
            nc.vector.tensor_sub(dR[:, 1:W], s[:, 1:W], s[:, 0 : W - 1])
            nc.vector.tensor_sub(dR[:, W : W + 1], auxU[:, 0:1], s[:, W - 1 : W])
            nc.vector.tensor_sub(dR[:, 0:1], s[:, 0:1], auxD[:, 1:2])
            nc.vector.memset(dR[0:1, 0:1], BIG)
            nc.vector.tensor_sub(d2[:, 2:W], s[:, 2:W], s[:, 0 : W - 2])
            nc.vector.tensor_sub(d2[:, W : W + 1], auxU[:, 0:1], s[:, W - 2 : W - 1])
            nc.vector.tensor_sub(d2[:, W + 1 : W + 2], auxU[:, 1:2], s[:, W - 1 : W])
            nc.vector.tensor_sub(d2[:, 0:2], s[:, 0:2], auxD[:, 0:2])
            nc.vector.memset(d2[0:1, 0:2], BIG)

            ca = dR[:, 1 : W + 1]   # R1
            cb = dR[:, 0:W]         # L1
            cc = d2[:, 2 : W + 2]   # R2
            cd = d2[:, 0:W]         # L2
            ab_lo = sp.tile([P, W], F32)
            ab_hi = sp.tile([P, W], F32)
            cd_lo = sp.tile([P, W], F32)
            cd_hi = sp.tile([P, W], F32)
            nc.vector.tensor_tensor(ab_lo[:, :], ca, cb, op=ALU.min)
            nc.vector.tensor_tensor(ab_hi[:, :], ca, cb, op=ALU.max)
            nc.vector.tensor_tensor(cd_lo[:, :], cc, cd, op=ALU.min)
            nc.vector.tensor_tensor(cd_hi[:, :], cc, cd, op=ALU.max)
            m1 = sp.tile([P, W], F32)
            mm = sp.tile([P, W], F32)
            nc.vector.tensor_tensor(m1[:, :], ab_lo[:, :], cd_lo[:, :], op=ALU.min)
            nc.vector.tensor_tensor(mm[:, :], ab_lo[:, :], cd_lo[:, :], op=ALU.max)
            nc.vector.tensor_tensor(ab_lo[:, :], ab_hi[:, :], cd_hi[:, :], op=ALU.min)
            nc.vector.tensor_tensor(mm[:, :], mm[:, :], ab_lo[:, :], op=ALU.min)
            nc.vector.tensor_add(m1[:, :], m1[:, :], mm[:, :])  # d1 + d2
            a3 = sp.tile([P, W], F32)
            nc.vector.tensor_scalar(
                a3[:, :], m1[:, :], 1.0 / 3.0, 2.0 * EPS, op0=ALU.mult, op1=ALU.add
            )
            nc.vector.reciprocal(out_sb[:, 0:W], a3[:, :])

            nc.sync.dma_start(out[:, :], out_sb[:, :])
    nc.finalize()
    return nc


_NC_CACHE = None


def _get_nc():
    global _NC_CACHE
    if _NC_CACHE is None:
        _NC_CACHE = _build()
    return _NC_CACHE


def make_in_maps(x_input, targets, w1, b1, w2, b2):
    x_input = np.ascontiguousarray(x_input, dtype=np.float32)
    targets = np.ascontiguousarray(targets, dtype=np.float32)
    w1 = np.ascontiguousarray(w1, dtype=np.float32)
    b1 = np.ascontiguousarray(b1, dtype=np.float32)
    w2 = np.ascontiguousarray(w2, dtype=np.float32)
    b2 = np.ascontiguousarray(b2, dtype=np.float32)
    c2n = (2.0 * w1.astype(np.float64) ** 2 * w2.astype(np.float64)).astype(np.float32)
    pidx = np.arange(P)

    def signs_col(k):
        return np.where((pidx & (k // W)) == 0, 1.0, -1.0).astype(np.float32)

    signs = np.stack([signs_col(k) for k in SGN_KS], axis=1).astype(np.float32)
    identity = np.eye(P, dtype=np.float32)
    shiftu = np.eye(P, P, -1, dtype=np.float32)  # auxU[m] = s[m+1]
    shiftd = np.eye(P, P, 1, dtype=np.float32)   # auxD[m] = s[m-1]
    in_maps = []
    for c in range(NCORES):
        xs = x_input[c * SHARD : (c + 1) * SHARD].reshape(NCH, P).T
        ts = targets[c * SHARD : (c + 1) * SHARD].reshape(NCH, P).T
        in_maps.append(
            {
                "x_full": x_input,
                "x_shard": np.ascontiguousarray(xs),
                "t_shard": np.ascontiguousarray(ts),
                "w1": w1,
                "b1": b1,
                "w2": w2,
                "b2": b2,
                "c2n": c2n,
                "signs": signs,
                "ident": identity,
                "shiftu": shiftu,
                "shiftd": shiftd,
            }
        )
    return in_maps


def kernel(x_input, targets, w1, b1, w2, b2, **_ignored):
    in_maps = make_in_maps(x_input, targets, w1, b1, w2, b2)
    nc = _get_nc()
    res = run_bass_kernel_spmd(nc, in_maps, core_ids=list(range(NCORES)))
    outs = [r["out"] for r in res.results]

    dens = outs[0][:, :W].astype(np.float64).ravel()
    sse = sum(o[:, W].astype(np.float64).sum() for o in outs)
    d2sq = sum(o[:, W + 1].astype(np.float64).sum() for o in outs)

    mse = sse / N
    mean_densn = (dens.sum() / N) / (dens.max() + EPS)
    penalty = 0.01 * (1.0 + 0.1 * mean_densn) * (d2sq / N)
    total = mse + penalty
    return np.array([total, mse, penalty], dtype=np.float32)
